# revision 34
# baseline (speedup 1.0000x reference)
"""BertCrossAttention (relative_key_query) Trainium2 kernel.

Full inputs -> full output. Sharding: 8 cores, core c handles batch b=c//2 and
heads [8*(c%2), 8*(c%2)+8). All sharding/slicing/transposition happens on the
host; each core runs an identical Bass program on its own slices.

Math (per core, per head h):
  q = xq @ Wq^T/8 + bq/8            [Lq=1024, 64]   (1/sqrt(64) pre-folded)
  k = x @ Wk^T + bk                  [Lk=2048, 64]
  v = x @ Wv^T + bv                  [Lk=2048, 64]
  S[l,r] = q.k + q.E[t] + k.E[t]/?? (E pre-scaled) + mask,  t = l - r + 2047
  out = softmax_r(S) @ v

Key layout trick: the key axis is REVERSED on the host (r' = 2047 - r), making
t = l + r'. Then with QE[l,t] = q[l].E[t] and KE[r',t] = k[r'].E[t]/8 stored
in DRAM as dense windowed blocks, both rel-score reads become plain 2D strided
DMAs (row stride = width+1 "skew trick" on flat DRAM):
  rel1[l, r'] = QE[l, l+r']   (read as [l-part, r'-free] tiles, then
                               PE-transpose-accumulated onto QK scores in PSUM)
  rel2^T[r', l] = KE[r', l+r'] (read directly as [r'-part, l-free] tiles)
Scores are built transposed (S^T [r'-part, l-free]) so PV needs no transpose
of the probabilities, and the softmax denominator comes from an extra
ones-column appended to V.
"""

import os
import sys
from contextlib import ExitStack

import numpy as np

sys.path.insert(0, "/opt/trn_rl_repo")

import concourse.bass as bass
import concourse.mybir as mybir
import concourse.tile as tile
from concourse import bacc
from concourse.masks import make_identity

F32 = mybir.dt.float32
F16 = mybir.dt.float16
BF16 = mybir.dt.bfloat16
USE_FP8 = os.environ.get("KQE_FP8", "1") == "1"
F8 = mybir.dt.float8e4 if USE_FP8 else mybir.dt.float16
ESCALE = 256.0 if USE_FP8 else 1.0  # QE/KE table scale (folded into E tables)

B, H, DH, D = 4, 16, 64, 1024
LQ, LK = 1024, 2048
HPC = 8              # heads per core
CH = HPC * DH        # 512 output channels per core
TW = 3072            # E-table columns used (t in [0, 3071))
QW = 2176            # QE block storage width (cols 0..2174 used, 2175 pad)
KW = 1151            # KE block storage width (cols 0..1150 used)
NKT = D // 128       # 8 contraction tiles for projections


def build_nc():
    nc = bacc.Bacc("TRN2", target_bir_lowering=False, debug=False, num_devices=8)

    xqT = nc.dram_tensor("xqT", [D, LQ], F16, kind="ExternalInput")
    xT = nc.dram_tensor("xT", [D, LK], F16, kind="ExternalInput")
    wqT = nc.dram_tensor("wqT", [D, CH], F16, kind="ExternalInput")
    wkT = nc.dram_tensor("wkT", [D, CH], F16, kind="ExternalInput")
    wvT = nc.dram_tensor("wvT", [D, CH], F16, kind="ExternalInput")
    bqv = nc.dram_tensor("bqv", [CH], F32, kind="ExternalInput")
    bkv = nc.dram_tensor("bkv", [CH], F32, kind="ExternalInput")
    bvv = nc.dram_tensor("bvv", [CH], F32, kind="ExternalInput")
    eT = nc.dram_tensor("eT", [DH, TW], F16, kind="ExternalInput")
    e8T = nc.dram_tensor("e8T", [DH, TW], F16, kind="ExternalInput")
    maskc = nc.dram_tensor("maskc", [128, 16], F32, kind="ExternalInput")
    out = nc.dram_tensor("out", [LQ, CH], F32, kind="ExternalOutput")

    with tile.TileContext(nc) as tc, ExitStack() as ctx:
        const = ctx.enter_context(tc.tile_pool(name="const", bufs=1))
        ident = const.tile([128, 128], F32)
        make_identity(nc, ident)

        et_sb = const.tile([DH, TW], F16, tag="et")
        e8t_sb = const.tile([DH, TW], F16, tag="e8t")
        nc.sync.dma_start(et_sb, eT[:, :])
        nc.sync.dma_start(e8t_sb, e8T[:, :])
        mask_sb = const.tile([128, 16], F32, tag="mask")
        nc.sync.dma_start(mask_sb, maskc[:, :])
        bq_sb = const.tile([128, 4], F32, tag="bq")
        bk_sb = const.tile([128, 4], F32, tag="bk")
        nc.sync.dma_start(bq_sb, bqv.rearrange("(t p) -> p t", p=128))
        nc.sync.dma_start(bk_sb, bkv.rearrange("(t p) -> p t", p=128))
        bv_sb = const.tile([128, CH], F32, tag="bv")
        nc.sync.dma_start(
            bv_sb, bass.AP(tensor=bvv, offset=0, ap=[[0, 128], [1, CH]])
        )

        # persistent per-core activation tensors
        persist = ctx.enter_context(tc.tile_pool(name="persist", bufs=1))
        v_sb = persist.tile([128, 16, HPC, DH + 1], BF16, tag="v")  # [r', j, h, dh|1]
        ctx_all = persist.tile([128, 8, CH], F32, tag="ctxo")  # [l%128, lblk, ch]
        nc.vector.memset(v_sb[:, :, :, DH], 1.0)

        # q/k projections land in DRAM scratch, read back per head
        qkd = ctx.enter_context(tc.tile_pool(name="qkd", bufs=1, space="DRAM"))
        qT_d = qkd.tile([CH, LQ], F16, tag="qTd")
        kT_d = qkd.tile([CH, LK], F16, tag="kTd")

        # ---------------- Phase 1: projections ----------------
        proj = ExitStack()
        ppool = proj.enter_context(tc.tile_pool(name="pp", bufs=8, space="PSUM"))
        pact = proj.enter_context(tc.tile_pool(name="pact", bufs=1))
        pst = proj.enter_context(tc.tile_pool(name="pst", bufs=3))
        if True:
            # Q projection: out[ch, l] ; lhsT = wqT k-tile, rhs = xqT k-tile
            xq_sb, wq_sb = [], []
            for t in range(NKT):
                xt_full = pact.tile([128, LK], F16, tag=f"x{t}")
                xt = xt_full[:, 0:LQ]
                wt = pact.tile([128, CH], F16, tag=f"w{t}")
                nc.sync.dma_start(xt, xqT[128 * t:128 * (t + 1), :])
                nc.sync.dma_start(wt, wqT[128 * t:128 * (t + 1), :])
                xq_sb.append(xt)
                wq_sb.append(wt)
            for m in range(4):          # ch tiles of 128
                for n in range(2):      # l chunks of 512
                    ps = ppool.tile([128, 512], F32, tag="pp")
                    for t in range(NKT):
                        nc.tensor.matmul(
                            ps,
                            wq_sb[t][:, 128 * m:128 * (m + 1)],
                            xq_sb[t][:, 512 * n:512 * (n + 1)],
                            start=(t == 0), stop=(t == NKT - 1),
                        )
                    st = pst.tile([128, 512], F16, tag="st")
                    nc.scalar.activation(
                        st, ps,
                        mybir.ActivationFunctionType.Identity,
                        bias=bq_sb[:, m:m + 1],
                    )
                    nc.sync.dma_start(
                        qT_d[128 * m:128 * (m + 1), 512 * n:512 * (n + 1)], st
                    )

        if True:
            x_sb = []
            for t in range(NKT):
                xt = pact.tile([128, LK], F16, tag=f"x{t}")
                nc.sync.dma_start(xt, xT[128 * t:128 * (t + 1), :])
                x_sb.append(xt)
            # K projection: out[ch, r']
            if True:
                wk_sb = []
                for t in range(NKT):
                    wt = pact.tile([128, CH], F16, tag=f"w{t}")
                    nc.sync.dma_start(wt, wkT[128 * t:128 * (t + 1), :])
                    wk_sb.append(wt)
                for m in range(4):
                    for n in range(4):      # r' chunks of 512
                        ps = ppool.tile([128, 512], F32, tag="pp")
                        for t in range(NKT):
                            nc.tensor.matmul(
                                ps,
                                wk_sb[t][:, 128 * m:128 * (m + 1)],
                                x_sb[t][:, 512 * n:512 * (n + 1)],
                                start=(t == 0), stop=(t == NKT - 1),
                            )
                        st = pst.tile([128, 512], F16, tag="st")
                        nc.scalar.activation(
                            st, ps,
                            mybir.ActivationFunctionType.Identity,
                            bias=bk_sb[:, m:m + 1],
                        )
                        nc.sync.dma_start(
                            kT_d[128 * m:128 * (m + 1), 512 * n:512 * (n + 1)], st
                        )
            # V projection, natural layout: out[r', ch]; lhsT = xT r'-slice
            if True:
                wv_sb = []
                for t in range(NKT):
                    wt = pact.tile([128, CH], F16, tag=f"w{t}")
                    nc.sync.dma_start(wt, wvT[128 * t:128 * (t + 1), :])
                    wv_sb.append(wt)
                for j in range(16):         # r' tiles of 128
                    ps = ppool.tile([128, CH], F32, tag="pp")
                    for t in range(NKT):
                        nc.tensor.matmul(
                            ps,
                            x_sb[t][:, 128 * j:128 * (j + 1)],
                            wv_sb[t],
                            start=(t == 0), stop=(t == NKT - 1),
                        )
                    for h in range(HPC):
                        nc.vector.tensor_add(
                            v_sb[:, j, h, 0:DH],
                            ps[:, DH * h:DH * (h + 1)],
                            bv_sb[:, DH * h:DH * (h + 1)],
                        )

        # ---------------- phase boundary ----------------
        # All engines observe the full clock once, so phase-2 first-touch
        # instructions inherit no released-zone semaphore waits (LDWEIGHTS has
        # limited sync-wait slots).
        proj.close()
        with tc.tile_critical():
            nc.all_engine_barrier()

        # ---------------- Phase 2: attention per head ----------------
        qe_dram = ctx.enter_context(tc.tile_pool(name="qed", bufs=16, space="DRAM"))
        ke_dram = ctx.enter_context(tc.tile_pool(name="ked", bufs=32, space="DRAM"))
        relp = ctx.enter_context(tc.tile_pool(name="relp", bufs=3))
        rel1p = ctx.enter_context(tc.tile_pool(name="rel1p", bufs=5))
        r2p = ctx.enter_context(tc.tile_pool(name="r2p", bufs=3))
        sp = ctx.enter_context(tc.tile_pool(name="sp", bufs=3))
        ptp = ctx.enter_context(tc.tile_pool(name="ptp", bufs=3))
        cnp = ctx.enter_context(tc.tile_pool(name="cnp", bufs=2))
        qeps = ctx.enter_context(tc.tile_pool(name="qeps", bufs=2, space="PSUM"))
        sps = ctx.enter_context(tc.tile_pool(name="sps", bufs=2, space="PSUM"))
        cps = ctx.enter_context(tc.tile_pool(name="cps", bufs=1, space="PSUM"))
        ctps = ctx.enter_context(tc.tile_pool(name="ctps", bufs=2, space="PSUM"))

        qkhp = ctx.enter_context(tc.tile_pool(name="qkhp", bufs=2))
        for h in range(HPC):
            # per-head base-0 tiles (matmul needs equal base partitions)
            qh = qkhp.tile([64, LQ], F16, tag="qh")
            kh = qkhp.tile([64, LK], F16, tag="kh")
            nc.sync.dma_start(qh, qT_d[64 * h:64 * h + 64, :])
            nc.sync.dma_start(kh, kT_d[64 * h:64 * h + 64, :])

            # QE blocks: QE[l, t], l-block i holds t-window [l0, l0+2175)
            qe_tiles = []
            for i in range(8):
                l0 = 128 * i
                qe_sb = relp.tile([128, QW], F8, tag="qe_sb")
                for c, w in ((0, 512), (512, 512), (1024, 512), (1536, 512), (2048, 127)):
                    ps = qeps.tile([128, 512], F32, tag="qeps")
                    nc.tensor.matmul(
                        ps[:, 0:w],
                        qh[:, l0:l0 + 128],
                        et_sb[:, l0 + c:l0 + c + w],
                        start=True, stop=True,
                    )
                    nc.vector.tensor_copy(qe_sb[:, c:c + w], ps[:, 0:w])
                qe_t = qe_dram.tile([128, QW], F8, tag="qe_d")
                nc.sync.dma_start(qe_t[:, 0:QW - 1], qe_sb[:, 0:QW - 1])
                qe_tiles.append(qe_t)

            # KE blocks: KE[r', t] + mask, r'-block j holds window [r0, r0+1151)
            ke_tiles = []
            for j in range(16):
                r0 = 128 * j
                ke_sb = relp.tile([128, KW], F8, tag="ke_sb")
                for c, w in ((0, 512), (512, 512), (1024, 127)):
                    ps = qeps.tile([128, 512], F32, tag="qeps")
                    nc.tensor.matmul(
                        ps[:, 0:w],
                        kh[:, r0:r0 + 128],
                        e8t_sb[:, r0 + c:r0 + c + w],
                        start=True, stop=True,
                    )
                    nc.scalar.activation(
                        ke_sb[:, c:c + w], ps[:, 0:w],
                        mybir.ActivationFunctionType.Identity,
                        bias=mask_sb[:, j:j + 1],
                    )
                ke_t = ke_dram.tile([128, KW], F8, tag="ke_d")
                nc.sync.dma_start(ke_t, ke_sb)
                ke_tiles.append(ke_t)

            # rel1 read-back: [l-part, r'-free] fp32 tiles via casting DMA.
            # QE block flat: addr(dl, c) = dl*QW + c ; rel1 needs c = dl + r'
            # -> addr = dl*(QW+1) + r'.
            rel1_sb = []
            for i in range(8):
                t1 = rel1p.tile([128, LK], F32, tag="rel1")
                src = bass.AP(
                    tensor=qe_tiles[i].tensor,
                    offset=qe_tiles[i].offset,
                    ap=[[QW + 1, 128], [1, LK]],
                )
                nc.gpsimd.dma_start(out=t1, in_=src)
                rel1_sb.append(t1)

            ctx_ps = cps.tile([DH + 1, LQ], F32, tag="ctxps")
            for lh in range(2):
                for j in range(16):
                    s_ps = sps.tile([128, 512], F32, tag="sps")
                    # QK^T: [r' 128, l 512]
                    nc.tensor.matmul(
                        s_ps,
                        kh[:, 128 * j:128 * (j + 1)],
                        qh[:, 512 * lh:512 * (lh + 1)],
                        start=True, stop=False,
                    )
                    # rel1: PE-transpose-accumulate 4 blocks of this l-half
                    for ii in range(4):
                        i = 4 * lh + ii
                        nc.tensor.matmul(
                            s_ps[:, 128 * ii:128 * (ii + 1)],
                            rel1_sb[i][:, 128 * j:128 * (j + 1)],
                            ident,
                            is_transpose=True,
                            start=False, stop=(ii == 3),
                        )
                    # rel2^T tile: KE flat addr(dr, c) = dr*KW + c, c = l + dr
                    # -> addr = dr*(KW+1) + l
                    r2 = r2p.tile([128, 512], F16, tag="r2")
                    src = bass.AP(
                        tensor=ke_tiles[j].tensor,
                        offset=ke_tiles[j].offset + 512 * lh,
                        ap=[[KW + 1, 128], [1, 512]],
                    )
                    nc.gpsimd.dma_start(out=r2, in_=src)
                    s_sb = sp.tile([128, 512], F32, tag="s_sb")
                    nc.vector.scalar_tensor_tensor(
                        out=s_sb, in0=r2, scalar=1.0, in1=s_ps,
                        op0=mybir.AluOpType.mult, op1=mybir.AluOpType.add,
                    )
                    # scores are carried at ESCALE x (q pre-scaled on host) so
                    # the fp8 QE/KE tables use the e4m3 normal range; divide
                    # back out inside the exp
                    pt = ptp.tile([128, 512], BF16, tag="pt")
                    nc.scalar.activation(
                        pt, s_sb, mybir.ActivationFunctionType.Exp,
                        scale=1.0 / ESCALE,
                    )
                    nc.tensor.matmul(
                        ctx_ps[:, 512 * lh:512 * (lh + 1)],
                        v_sb[:, j, h, :],
                        pt,
                        start=(j == 0), stop=(j == 15),
                    )

            # copy ctx+rowsum to SBUF; 1/rowsum is applied per-partition after
            # the transpose (ACT copy with per-partition scale)
            cn_sb = cnp.tile([DH + 1, LQ], F32, tag="ctxn")
            nc.vector.tensor_copy(cn_sb, ctx_ps)
            for i in range(8):
                ct = ctps.tile([128, DH + 1], F32, tag="ctps")
                nc.tensor.matmul(
                    ct,
                    cn_sb[:, 128 * i:128 * (i + 1)],
                    ident[0:DH + 1, 0:DH + 1],
                    is_transpose=True,
                    start=True, stop=True,
                )
                rs_inv = cnp.tile([128, 1], F32, tag="rsinv")
                nc.vector.reciprocal(rs_inv, ct[:, DH:DH + 1])
                nc.scalar.activation(
                    ctx_all[:, i, DH * h:DH * (h + 1)], ct[:, 0:DH],
                    mybir.ActivationFunctionType.Copy,
                    scale=rs_inv,
                )

        nc.sync.dma_start(out.rearrange("(i p) c -> p i c", p=128), ctx_all)

    nc.compile()
    return nc


def make_in_maps(inputs):
    hs = np.asarray(inputs["hidden_states"], np.float32)
    qhs = np.asarray(inputs["query_hidden_states"], np.float32)
    am = np.asarray(inputs["attention_mask"], np.float32)
    Wq = np.asarray(inputs["Wq"], np.float32)
    bq = np.asarray(inputs["bq"], np.float32)
    Wk = np.asarray(inputs["Wk"], np.float32)
    bk = np.asarray(inputs["bk"], np.float32)
    Wv = np.asarray(inputs["Wv"], np.float32)
    bv = np.asarray(inputs["bv"], np.float32)
    de = np.asarray(inputs["dist_emb"], np.float32)

    # All scores are carried at ESCALE x: q is pre-scaled by ESCALE (via Wq,
    # bq) which covers the QK and q.E terms; the k.E term gets ESCALE via its
    # E table. The exp divides ESCALE back out. This puts the fp8-stored
    # QE/KE tables in e4m3's normal range.
    eT = np.zeros((DH, TW), np.float32)
    eT[:, :3071] = de[:3071].T
    e8T = (eT / 8.0 * ESCALE).astype(np.float32)

    F16_KEYS = {"xqT", "xT", "wqT", "wkT", "wvT", "eT", "e8T"}
    in_maps = []
    for core in range(8):
        b = core // 2
        hg = core % 2
        sl = slice(CH * hg, CH * (hg + 1))
        m = {
            "xqT": np.ascontiguousarray(qhs[b].T),
            "xT": np.ascontiguousarray(hs[b].T[:, ::-1]),
            "wqT": np.ascontiguousarray(Wq[sl].T) * (ESCALE / 8.0),
            "wkT": np.ascontiguousarray(Wk[sl].T),
            "wvT": np.ascontiguousarray(Wv[sl].T),
            "bqv": np.ascontiguousarray(bq[sl]) * (ESCALE / 8.0),
            "bkv": np.ascontiguousarray(bk[sl]),
            "bvv": np.ascontiguousarray(bv[sl]),
            "eT": eT,
            "e8T": e8T,
            "maskc": np.ascontiguousarray(am[b, 0, 0, ::-1].reshape(16, 128).T) * ESCALE,
        }
        in_maps.append({
            k: np.ascontiguousarray(
                v.astype(np.float16 if k in F16_KEYS else np.float32)
            )
            for k, v in m.items()
        })
    return in_maps


_CACHED = {}


def assemble_output(per_core_results):
    out = np.zeros((B, LQ, D), np.float32)
    for core in range(8):
        b = core // 2
        hg = core % 2
        out[b, :, CH * hg:CH * (hg + 1)] = per_core_results[core]["out"]
    return out


def kernel(**inputs):
    from concourse.bass_utils import run_bass_kernel_spmd

    if "nc" not in _CACHED:
        _CACHED["nc"] = build_nc()
    nc = _CACHED["nc"]
    in_maps = make_in_maps(inputs)
    res = run_bass_kernel_spmd(nc, in_maps, list(range(8)))
    _CACHED["last_result"] = res
    return assemble_output(res.results)

